# revision 1
# baseline (speedup 1.0000x reference)
"""Trainium2 Bass kernel for implicit cross-attention (keys/values = queries + 1 ctx token).

Sharding: 8 cores = 4 batches x 2 head-groups (8 heads each). Each core computes
q = x_b @ Wq[:, g], causal flash-style attention over keys [ctx, q_0..q_{N-1}],
and a partial output projection out @ Wo[g, :]. Host sums the two head-group
partials per batch and adds the bias.

Attention is processed per head-pair (two heads sharing a 128-partition q^T
tile) and per 1024-query half, so both heads' score matmuls are independent
in-flight PE work while the other head's exp runs on ScalarE.
"""

import numpy as np

import concourse.bass as bass
import concourse.mybir as mybir
from concourse import bacc
from concourse.tile import TileContext
from concourse.bass_utils import run_bass_kernel_spmd
from concourse.masks import make_identity

FP = mybir.dt.float32
FPR = mybir.dt.float32r
BF = mybir.dt.bfloat16

N = 2048          # sequence length
CD = 1024         # model dim
HD = 512          # head-dim cols per core (8 heads x 64)
D = 64            # dim per head
NHEAD = 8         # heads per core
SCALE = 0.125     # D ** -0.5
NMT = HD // 128   # 4 hd chunks of 128 (2 heads each)
NCC = CD // 128   # 8 contraction chunks
NIC = N // 512    # 4 query chunks of 512
NKB = N // 128    # 16 key blocks of 128

USE_FPR = True    # float32r (full-rate PE fp32) for all matmuls
MMDT = FPR if USE_FPR else FP


def _f32(ap):
    return ap.bitcast(FP) if USE_FPR else ap


def _bc(ap):
    return ap.bitcast(FPR) if USE_FPR else ap


def _build_nc():
    nc = bacc.Bacc("TRN2", target_bir_lowering=False)
    x_d = nc.declare_dram_parameter("x", [N, CD], FP, isOutput=False)
    wq_d = nc.declare_dram_parameter("wq", [CD, HD], FP, isOutput=False)
    wk_d = nc.declare_dram_parameter("wk", [CD, HD], FP, isOutput=False)
    wv_d = nc.declare_dram_parameter("wv", [CD, HD], FP, isOutput=False)
    wo_d = nc.declare_dram_parameter("wo", [HD, CD], FP, isOutput=False)
    ctx_d = nc.declare_dram_parameter("ctx", [1, CD], FP, isOutput=False)
    y_d = nc.declare_dram_parameter("y", [N, CD], FP, isOutput=True)

    with TileContext(nc) as tc, tc.tile_pool(name="persist", bufs=1) as pp:
        # ---- persistent SBUF tensors (one slot per tag) ----
        ident = pp.tile([128, 128], FP, tag="ident", name="ident")
        # two stacked 64x64 identities (for transposes of tiles based at partition 64)
        ident2 = pp.tile([128, 64], FP, tag="ident2", name="ident2")
        ones11 = pp.tile([1, 1], FP, tag="ones11", name="ones11")
        ones16 = pp.tile([128, 16], FP, tag="ones16", name="ones16")
        zeros16 = pp.tile([128, 16], FP, tag="zeros16", name="zeros16")
        zpad64 = pp.tile([128, D], FP, tag="zpad64", name="zpad64")
        tri = pp.tile([128, 128], FP, tag="tri", name="tri")
        ctxT_sb = pp.tile([128, NCC], FP, tag="ctxT_sb", name="ctxT_sb")
        kctx_sb = pp.tile([1, HD], FP, tag="kctx_sb", name="kctx_sb")
        kct_sb = pp.tile([64, NHEAD], MMDT, tag="kct_sb", name="kct_sb")
        # zero-padded k_ctx^T columns per head pair (K=128 ctx score matmuls)
        kct2 = pp.tile([128, NHEAD], MMDT, tag="kct2", name="kct2")
        vctx_row = pp.tile([65, NHEAD * (D + 1)], BF, tag="vctx_row", name="vctx_row")
        qkT = [pp.tile([128, N], MMDT, tag=f"qkT{m}", name=f"qkT{m}") for m in range(NMT)]
        # pcx packed per pair: even head at partition 0, odd head at partition 64
        pcx_all = [pp.tile([65, N], BF, tag=f"pcx{m}", name=f"pcx{m}") for m in range(NMT)]
        # zero-banded q^T copies per head (K=128 score matmuls: the other head's
        # 64-partition band is zero, so the full-tile rhs contraction is exact)
        qkZ = [pp.tile([128, N], MMDT, tag=f"qkZ{h}", name=f"qkZ{h}") for h in range(NHEAD)]
        wo_sb = [pp.tile([128, CD], MMDT, tag=f"wo_sb{m}", name=f"wo_sb{m}") for m in range(NMT)]

        make_identity(nc, ident)
        nc.gpsimd.memset(ident2, 0.0)
        # ident2[p, f] = 1 where p == f or p == f + 64
        nc.gpsimd.affine_select(
            out=ident2, in_=ident2, compare_op=mybir.AluOpType.not_equal,
            fill=1.0, base=0, pattern=[[-1, 64]], channel_multiplier=1)
        nc.gpsimd.affine_select(
            out=ident2, in_=ident2, compare_op=mybir.AluOpType.not_equal,
            fill=1.0, base=-64, pattern=[[-1, 64]], channel_multiplier=1)
        nc.vector.memset(ones11, 1.0)
        nc.vector.memset(ones16, 1.0)
        nc.vector.memset(zeros16, 0.0)
        nc.vector.memset(zpad64, 0.0)
        nc.vector.memset(tri, 1.0)
        nc.gpsimd.affine_select(
            out=tri, in_=tri, compare_op=mybir.AluOpType.is_ge,
            fill=0.0, base=0, pattern=[[1, 128]], channel_multiplier=-1)

        # ---- weight / input DMA ----
        for m in range(NMT):
            nc.sync.dma_start(wo_sb[m], _bc(wo_d[128 * m:128 * (m + 1), :]))

        with tc.tile_pool(name="qp", bufs=2, space="PSUM") as qp_pool, \
             tc.tile_pool(name="tp", bufs=2, space="PSUM") as tp_pool, \
             tc.tile_pool(name="wkv", bufs=2) as wkv_pool, \
             tc.tile_pool(name="xn", bufs=4) as xn_pool, \
             tc.tile_pool(name="bwq", bufs=1) as bwq_pool:

            wq_sb = [bwq_pool.tile([128, HD], MMDT, tag=f"wq_sb{c}", name=f"wq_sb{c}") for c in range(NCC)]
            xT_sb = [bwq_pool.tile([128, 512], MMDT, tag=f"xT_sb{c}", name=f"xT_sb{c}") for c in range(NCC)]
            ctx_sb = bwq_pool.tile([1, CD], FP, tag="ctx_sb", name="ctx_sb")
            nc.sync.dma_start(ctx_sb, ctx_d[0:1, :])
            zwide = bwq_pool.tile([128, N], FP, tag="zwide", name="zwide")
            nc.vector.memset(zwide, 0.0)
            for h in range(NHEAD):
                band = 64 * (h % 2)
                nc.vector.tensor_copy(qkZ[h][64 - band:128 - band, :], zwide[0:64, :])

            for c in range(NCC):
                nc.sync.dma_start(wq_sb[c], _bc(wq_d[128 * c:128 * (c + 1), :]))

            # ---- context k/v projections ----
            # ctx^T via K=1 matmuls: out[128,1] = ctx_chunk.T @ ones
            ctxT_ps = qp_pool.tile([128, 512], FP, tag="qp")
            for c in range(NCC):
                nc.tensor.matmul(ctxT_ps[:, c:c + 1], ctx_sb[0:1, 128 * c:128 * (c + 1)],
                                 ones11, start=True, stop=True)
            nc.vector.tensor_copy(ctxT_sb, ctxT_ps[:, 0:NCC])

            # k_ctx / v_ctx as natural rows: out[1, 512] = ctx_chunk.T @ W chunk
            kv_ps = qp_pool.tile([128, 1024], FP, tag="qp")
            for (w_d, base) in ((wk_d, 0), (wv_d, 512)):
                for c in range(NCC):
                    wt = wkv_pool.tile([128, HD], FP, tag="wkv")
                    nc.sync.dma_start(wt, w_d[128 * c:128 * (c + 1), :])
                    nc.tensor.matmul(kv_ps[0:1, base:base + 512],
                                     ctxT_sb[:, c:c + 1], wt,
                                     start=(c == 0), stop=(c == NCC - 1))
            nc.vector.tensor_copy(kctx_sb, kv_ps[0:1, 0:512])
            nc.vector.tensor_copy(
                vctx_row[0:1, :].rearrange("p (h e) -> p h e", e=D + 1)[:, :, 0:D],
                kv_ps[0:1, 512:512 + NHEAD * D].rearrange("p (h e) -> p h e", e=D))
            nc.vector.tensor_copy(
                vctx_row[0:1, :].rearrange("p (h e) -> p h e", e=D + 1)[:, :, D:D + 1],
                ones16[0:1, 0:NHEAD])
            nc.sync.dma_start(vctx_row[64:65, :], vctx_row[0:1, :])

            # k_ctx^T per head (transpose lands at partition 0; SBUF->SBUF DMA
            # shifts odd heads to the 64-partition band of kct2)
            kct_ps = tp_pool.tile([128, 512], FP, tag="tp")
            for h in range(NHEAD):
                nc.tensor.transpose(kct_ps[0:64, h:h + 1],
                                    kctx_sb[0:1, 64 * h:64 * h + 64], ones11)
            nc.vector.tensor_copy(kct_sb, kct_ps[0:64, 0:NHEAD])
            nc.vector.tensor_copy(kct2, zeros16[:, 0:NHEAD])
            for h in range(NHEAD):
                if h % 2 == 0:
                    nc.vector.tensor_copy(kct2[0:64, h:h + 1], kct_sb[:, h:h + 1])
                else:
                    nc.sync.dma_start(kct2[64:128, h:h + 1], kct_sb[:, h:h + 1])

            # ---- x^T and q^T (per 512-query chunk) ----
            for ic in range(NIC):
                xnat = []
                for s in range(4):
                    xt = xn_pool.tile([128, CD], FP, tag="xn")
                    nc.sync.dma_start(xt, x_d[512 * ic + 128 * s: 512 * ic + 128 * (s + 1), :])
                    xnat.append(xt)
                for c in range(NCC):
                    tps = tp_pool.tile([128, 512], FP, tag="tp")
                    for s in range(4):
                        nc.tensor.transpose(tps[:, 128 * s:128 * (s + 1)],
                                            xnat[s][:, 128 * c:128 * (c + 1)], ident)
                    nc.vector.tensor_copy(xT_sb[c], tps)
                for m in range(NMT):
                    qps = qp_pool.tile([128, 512], FP, tag="qp")
                    for c in range(NCC):
                        nc.tensor.matmul(qps,
                                         wq_sb[c][:, 128 * m:128 * (m + 1)],
                                         xT_sb[c],
                                         start=(c == 0), stop=(c == NCC - 1))
                    nc.vector.tensor_copy(qkT[m][:, 512 * ic: 512 * (ic + 1)], qps)
                    nc.scalar.copy(qkZ[2 * m][0:64, 512 * ic: 512 * (ic + 1)], qps[0:64, :])
                    nc.scalar.copy(qkZ[2 * m + 1][64:128, 512 * ic: 512 * (ic + 1)], qps[64:128, :])

        # ---- ctx score rows for all heads (overlaps projection tail; odd
        # heads' rows are DMA-shifted to partition 64 of the pair tile) ----
        with tc.tile_pool(name="scp", bufs=2, space="PSUM") as scp_pool, \
             tc.tile_pool(name="pct", bufs=2) as pct_pool:
            for m in range(NMT):
                for hi in range(2):
                    h = 2 * m + hi
                    sc = scp_pool.tile([1, N], FP, tag="scp", name="sc")
                    for s in range(4):
                        nc.tensor.matmul(sc[0:1, 512 * s:512 * (s + 1)],
                                         kct2[:, h:h + 1],
                                         qkT[m][:, 512 * s:512 * (s + 1)],
                                         start=True, stop=True)
                    if hi == 0:
                        nc.scalar.activation(pcx_all[m][0:1, :], sc,
                                             mybir.ActivationFunctionType.Exp, scale=SCALE)
                    else:
                        ptmp = pct_pool.tile([1, N], BF, tag="pct", name="ptmp")
                        nc.scalar.activation(ptmp, sc,
                                             mybir.ActivationFunctionType.Exp, scale=SCALE)
                        nc.sync.dma_start(pcx_all[m][64:65, :], ptmp)

        # ---- attention ----
        att2_pool = tc.alloc_tile_pool(name="att2", bufs=1)
        attnT = [att2_pool.tile([128, N], MMDT, tag=f"attnT{m}", name=f"attnT{m}") for m in range(NMT)]
        with tc.tile_pool(name="ps", bufs=2, space="PSUM") as ps_pool, \
             tc.tile_pool(name="pu", bufs=1, space="PSUM") as pu_pool, \
             tc.tile_pool(name="psb", bufs=3) as psb_pool, \
             tc.tile_pool(name="rc", bufs=1) as rc_pool, \
             tc.tile_pool(name="vsbp", bufs=1) as vsb_pool, \
             tc.tile_pool(name="usb", bufs=2) as usb_pool:
            for m in range(NMT):
                heads = (2 * m, 2 * m + 1)
                bands = (0, 64)
                vsb = {}

                # v_aug: transpose q^T -> natural, packed 8 blocks per PSUM tile
                for hi in range(2):
                    h, band = heads[hi], bands[hi]
                    # flat [128, 16*65 + 64pad]; U reads 128-wide windows so the
                    # stationary is always full M=128 (tail cols land in unread
                    # PSUM rows 65..127)
                    vsb[h] = vsb_pool.tile([128, NKB * (D + 1) + D], MMDT,
                                           tag=f"vs{hi}", name=f"vs{hi}")
                    vs3 = vsb[h][:, 0:NKB * (D + 1)].rearrange("p (a b) -> p a b", b=D + 1)
                    for g in range(2):
                        vt_ps = ps_pool.tile([128, 1024], FP, tag="ps")
                        for j in range(8):
                            kb = 8 * g + j
                            nc.tensor.transpose(
                                vt_ps[:, 64 * j:64 * (j + 1)],
                                _f32(qkZ[h][band:band + 64, 128 * kb: 128 * (kb + 1)]),
                                ident2[band:band + 64, 0:64])
                        nc.vector.tensor_copy(
                            vs3[:, 8 * g:8 * (g + 1), 0:D],
                            vt_ps[:, 0:512].rearrange("p (j e) -> p j e", e=D))
                    nc.vector.tensor_copy(vs3[:, :, D:D + 1], ones16[:, 0:NKB])
                    nc.vector.tensor_copy(vsb[h][:, NKB * (D + 1):], zpad64)

                # full query range per head: each stationary (keys / v-block)
                # serves up to 4 consecutive matmuls
                for hi in range(2):
                    h, band = heads[hi], bands[hi]
                    qh = qkT[m][band:band + 64, :]
                    pu = pu_pool.tile([128, N], FP, tag="puf", name="puf")

                    # ctx contribution: K=1 outer products seed each U region
                    for s in range(NIC):
                        nc.tensor.matmul(pu[0:65, 512 * s:512 * (s + 1)],
                                         vctx_row[band:band + 1, 65 * h:65 * h + 65],
                                         pcx_all[m][band:band + 1, 512 * s:512 * (s + 1)],
                                         start=True, stop=False)

                    for kb in range(1, NKB + 1):
                        i0 = 128 * (kb - 1)          # first query that sees this block
                        keys = qkZ[h][:, 128 * (kb - 1): 128 * kb]
                        spg = {}
                        # scores: one stationary, all chunks
                        for g in range(i0 // 1024, 2):
                            lo = max(i0, 1024 * g)
                            sp = ps_pool.tile([128, 1024], FP, tag="ps")
                            spg[g] = (sp, lo)
                            q0 = lo
                            while q0 < 1024 * (g + 1):
                                q1 = min(512 * (q0 // 512 + 1), 1024 * (g + 1))
                                o = q0 - 1024 * g
                                nc.tensor.matmul(sp[:, o:o + (q1 - q0)],
                                                 keys, qkT[m][:, q0:q1],
                                                 start=True, stop=True)
                                q0 = q1
                        ptg = {}
                        for g, (sp, lo) in spg.items():
                            off = lo - 1024 * g
                            pt = psb_pool.tile([128, 1024], MMDT, tag="psb")
                            ptg[g] = pt
                            nc.scalar.activation(pt[:, off:1024], sp[:, off:1024],
                                                 mybir.ActivationFunctionType.Exp,
                                                 scale=SCALE)
                            if lo == i0:
                                # mask cols [i0, i0+128): keep where icol >= key row
                                nc.vector.tensor_mul(pt[:, off:off + 128],
                                                     pt[:, off:off + 128], tri)
                        # U: one stationary, all chunks
                        for g, (sp, lo) in spg.items():
                            q0 = lo
                            while q0 < 1024 * (g + 1):
                                q1 = min(512 * (q0 // 512 + 1), 1024 * (g + 1))
                                o = q0 - 1024 * g
                                nc.tensor.matmul(pu[:, q0:q1],
                                                 vsb[h][:, 65 * (kb - 1): 65 * (kb - 1) + 128],
                                                 ptg[g][:, o:o + (q1 - q0)],
                                                 start=False, stop=(kb == (q0 // 512) * 4 + 4))
                                q0 = q1

                    # normalize: attnT = U[0:64] / U[64]. Copy U out of PSUM
                    # first so the pu slot frees fast; recip chain runs off the
                    # critical path from SBUF.
                    u_sb = usb_pool.tile([65, N], FP, tag="usb", name="usb")
                    nc.vector.tensor_copy(u_sb, pu[0:65, :])
                    for s in range(NIC):
                        sl = slice(512 * s, 512 * (s + 1))
                        recip_sb = rc_pool.tile([1, 512], FP, tag="rcs")
                        recip_bc = rc_pool.tile([64, 512], FP, tag="rcb")
                        nc.vector.reciprocal(recip_sb, u_sb[64:65, sl])
                        nc.gpsimd.partition_broadcast(recip_bc, recip_sb)
                        nc.vector.tensor_mul(attnT[m][band:band + 64, sl],
                                             u_sb[0:64, sl], recip_bc)

        # ---- output projection ----
        with tc.tile_pool(name="py", bufs=2, space="PSUM") as py_pool, \
             tc.tile_pool(name="ysb", bufs=2) as y_pool:
            for nb in range(N // 128):
                py = py_pool.tile([128, CD], FP, tag="py")
                for co in range(2):
                    for m in range(NMT):
                        nc.tensor.matmul(py[:, 512 * co:512 * (co + 1)],
                                         attnT[m][:, 128 * nb:128 * (nb + 1)],
                                         wo_sb[m][:, 512 * co:512 * (co + 1)],
                                         start=(m == 0), stop=(m == NMT - 1))
                ysb = y_pool.tile([128, CD], FP, tag="ysb")
                nc.vector.tensor_copy(ysb, py)
                nc.sync.dma_start(y_d[128 * nb:128 * (nb + 1), :], ysb)
        att2_pool.release()

    nc.compile()
    return nc


_NC = None


def _get_nc():
    global _NC
    if _NC is None:
        _NC = _build_nc()
    return _NC


def _shard(inputs):
    x = np.ascontiguousarray(np.asarray(inputs["x"], dtype=np.float32))
    context = np.ascontiguousarray(np.asarray(inputs["context"], dtype=np.float32))
    Wq = np.asarray(inputs["Wq"], dtype=np.float32)
    Wk = np.asarray(inputs["Wk"], dtype=np.float32)
    Wv = np.asarray(inputs["Wv"], dtype=np.float32)
    Wo = np.asarray(inputs["Wo"], dtype=np.float32)
    in_maps = []
    for c in range(8):
        b, g = c // 2, c % 2
        sl = slice(HD * g, HD * (g + 1))
        in_maps.append({
            "x": np.ascontiguousarray(x[b]),
            "wq": np.ascontiguousarray(Wq[:, sl]),
            "wk": np.ascontiguousarray(Wk[:, sl]),
            "wv": np.ascontiguousarray(Wv[:, sl]),
            "wo": np.ascontiguousarray(Wo[sl, :]),
            "ctx": np.ascontiguousarray(context[b:b + 1]),
        })
    return in_maps


def _run(inputs, trace=False, **kw):
    nc = _get_nc()
    in_maps = _shard(inputs)
    res = run_bass_kernel_spmd(nc, in_maps, list(range(8)), trace=trace, **kw)
    bo = np.asarray(inputs["bo"], dtype=np.float32)
    B = np.asarray(inputs["x"]).shape[0]
    y = np.empty((B, N, CD), dtype=np.float32)
    for b in range(B):
        y[b] = res.results[2 * b]["y"] + res.results[2 * b + 1]["y"] + bo
    return y, res


def kernel(**inputs):
    y, _ = _run(inputs)
    return y



# revision 17
# speedup vs baseline: 1.5341x; 1.5341x over previous
"""Trainium2 Bass kernel for implicit cross-attention (keys/values = queries + 1 ctx token).

Sharding: 8 cores = 4 batches x 2 head-groups (8 heads each). Each core computes
q = x_b @ Wq[:, g], causal attention over keys [ctx, q_0..q_{N-1}], and a
partial output projection out @ Wo[g, :]. Host sums the two head-group partials
per batch and adds the bias.

Schedule: queries processed in two 1024-column chunks. Per chunk: projection
(PE transposes + matmuls), ctx-score rows (batched exp), v-block build, then
attention per head with a software-pipelined score->exp->U skew so the
ScalarE exp stream stays off the PE critical path. The output projection of
chunk 0 is emitted after attention of chunk 1 so it fills PE gaps while
ScalarE drains exps; chunk 1's projection likewise overlaps attention of
chunk 0 via the Tile list scheduler.
"""

import numpy as np

import concourse.bass as bass
import concourse.mybir as mybir
from concourse import bacc
from concourse.tile import TileContext
from concourse.bass_utils import run_bass_kernel_spmd
from concourse.masks import make_identity

FP = mybir.dt.float32
FPR = mybir.dt.float32r
BF = mybir.dt.bfloat16

N = 2048          # sequence length
CD = 1024         # model dim
HD = 512          # head-dim cols per core (8 heads x 64)
D = 64            # dim per head
NHEAD = 8         # heads per core
SCALE = 0.125     # D ** -0.5
NMT = HD // 128   # 4 head pairs (2 heads per 128-partition tile)
NCC = CD // 128   # 8 contraction chunks
NKB = N // 128    # 16 key blocks of 128
QC = 1024         # query chunk width
NQC = N // QC     # 2 chunks
VW = NKB * (D + 1) + D  # vsb flat width (1104): last U window needs 64 pad


def _spans(lo, hi):
    """Split [lo, hi) on the 512 grid: each matmul's PSUM output must stay
    within one 2KB bank (512 fp32 columns)."""
    out = []
    q0 = lo
    while q0 < hi:
        q1 = min(hi, (q0 // 512 + 1) * 512)
        out.append((q0, q1))
        q0 = q1
    return out


def _build_nc():
    nc = bacc.Bacc("TRN2", target_bir_lowering=False)
    x_d = nc.declare_dram_parameter("x", [N, CD], FP, isOutput=False)
    wq_d = nc.declare_dram_parameter("wq", [CD, HD], FP, isOutput=False)
    wk_d = nc.declare_dram_parameter("wk", [CD, HD], FP, isOutput=False)
    wv_d = nc.declare_dram_parameter("wv", [CD, HD], FP, isOutput=False)
    wo_d = nc.declare_dram_parameter("wo", [HD, CD], FP, isOutput=False)
    ctx_d = nc.declare_dram_parameter("ctx", [1, CD], FP, isOutput=False)
    y_d = nc.declare_dram_parameter("y", [N, CD], FP, isOutput=True)

    with TileContext(nc) as tc, \
         tc.tile_pool(name="pp", bufs=1) as pp, \
         tc.tile_pool(name="spp", bufs=2, space="PSUM") as sp_pool, \
         tc.tile_pool(name="pup", bufs=1, space="PSUM") as pu_pool, \
         tc.tile_pool(name="pjp", bufs=2, space="PSUM") as pj_pool, \
         tc.tile_pool(name="xnp", bufs=4) as xn_pool, \
         tc.tile_pool(name="wsp", bufs=2) as ws_pool, \
         tc.tile_pool(name="ptp", bufs=3) as pt_pool, \
         tc.tile_pool(name="usp", bufs=2) as us_pool, \
         tc.tile_pool(name="rcp", bufs=2) as rc_pool, \
         tc.tile_pool(name="ysp", bufs=3) as ys_pool:

        # ---- persistent SBUF tensors ----
        ident = pp.tile([128, 128], FP, tag="ident", name="ident")
        ident2 = pp.tile([128, 64], FP, tag="ident2", name="ident2")
        tri = pp.tile([128, 128], BF, tag="tri", name="tri")
        ones11 = pp.tile([1, 1], FP, tag="ones11", name="ones11")
        ones16 = pp.tile([128, 16], BF, tag="ones16", name="ones16")
        ctx_sb = pp.tile([1, CD], FP, tag="ctx_sb", name="ctx_sb")
        ctxT_sb = pp.tile([128, NCC], FPR, tag="ctxT_sb", name="ctxT_sb")
        zeros8 = pp.tile([128, NHEAD], FP, tag="zeros8", name="zeros8")
        identR = pp.tile([128, 128], FPR, tag="identR", name="identR")
        id2R = pp.tile([128, 64], FPR, tag="id2R", name="id2R")
        kctx_sb = pp.tile([1, HD], FP, tag="kctx_sb", name="kctx_sb")
        kct_sb = pp.tile([64, NHEAD], FPR, tag="kct_sb", name="kct_sb")
        kct2 = pp.tile([128, NHEAD], FPR, tag="kct2", name="kct2")
        # per-head v_ctx stationary padded to 128 cols so the ctx seed matmul
        # starts the full PSUM partition range the U matmuls accumulate into
        vctx_row = pp.tile([65, NHEAD * 128], BF, tag="vctx", name="vctx")
        # per (chunk, pair): q^T fp32r and attn^T bf16
        qkT = [[pp.tile([128, QC], FPR, tag=f"qkT{c}_{m}", name=f"qkT{c}_{m}")
                for m in range(NMT)] for c in range(NQC)]
        attnT = [[pp.tile([128, QC], BF, tag=f"at{c}_{m}", name=f"at{c}_{m}")
                  for m in range(NMT)] for c in range(NQC)]
        # ctx-score rows: heads of pair m at partitions 0 / 64
        pcx = [pp.tile([65, N], BF, tag=f"pcx{m}", name=f"pcx{m}") for m in range(NMT)]
        vsb = [pp.tile([128, VW], BF, tag=f"vsb{h}", name=f"vsb{h}") for h in range(NHEAD)]
        wq_sb = [pp.tile([128, HD], FPR, tag=f"wq{c}", name=f"wq{c}") for c in range(NCC)]
        wo_sb = [pp.tile([128, CD], BF, tag=f"wo{m}", name=f"wo{m}") for m in range(NMT)]
        xT_sb = [pp.tile([128, 512], FPR, tag=f"xT{c}", name=f"xT{c}") for c in range(NCC)]

        make_identity(nc, ident)
        nc.gpsimd.memset(ident2, 0.0)
        # ident2[p, f] = 1 where p == f or p == f + 64 (stacked 64x64 identities)
        nc.gpsimd.affine_select(
            out=ident2, in_=ident2, compare_op=mybir.AluOpType.not_equal,
            fill=1.0, base=0, pattern=[[-1, 64]], channel_multiplier=1)
        nc.gpsimd.affine_select(
            out=ident2, in_=ident2, compare_op=mybir.AluOpType.not_equal,
            fill=1.0, base=-64, pattern=[[-1, 64]], channel_multiplier=1)
        nc.vector.memset(ones11, 1.0)
        nc.vector.memset(ones16, 1.0)
        nc.vector.memset(zeros8, 0.0)
        nc.vector.tensor_copy(identR, ident)
        nc.vector.tensor_copy(id2R, ident2)
        nc.vector.memset(tri, 1.0)
        # keep pt[key p, query f] where f >= p (causal within diagonal block)
        nc.gpsimd.affine_select(
            out=tri, in_=tri, compare_op=mybir.AluOpType.is_ge,
            fill=0.0, base=0, pattern=[[1, 128]], channel_multiplier=-1)
        for h in range(NHEAD):
            nc.gpsimd.memset(vsb[h], 0.0)

        # ---- weight / ctx DMA ----
        nc.sync.dma_start(ctx_sb, ctx_d[0:1, :])
        for c in range(NCC):
            nc.sync.dma_start(wq_sb[c], wq_d[128 * c:128 * (c + 1), :].bitcast(FPR))
        for m in range(NMT):
            wot = ws_pool.tile([128, CD], FP, tag="ws", name="wot")
            nc.sync.dma_start(wot, wo_d[128 * m:128 * (m + 1), :])
            nc.vector.tensor_copy(wo_sb[m], wot)

        # ---- context k/v projections (tiny) ----
        # ctx^T via K=1 matmuls: out[128,1] = ctx_chunk.T @ ones
        ctxT_ps = pj_pool.tile([128, 512], FP, tag="pj", name="ctxT_ps")
        for c in range(NCC):
            nc.tensor.matmul(ctxT_ps[:, c:c + 1], ctx_sb[0:1, 128 * c:128 * (c + 1)],
                             ones11, start=True, stop=True)
        nc.vector.tensor_copy(ctxT_sb, ctxT_ps[:, 0:NCC])

        # k_ctx / v_ctx rows: out[1, 512] = ctx_chunk.T @ W chunk
        for wi, w_d in enumerate((wk_d, wv_d)):
            kv_ps = pj_pool.tile([128, 512], FP, tag="pj", name="kv_ps")
            for c in range(NCC):
                wt = ws_pool.tile([128, HD], FPR, tag="wskv", name="wt")
                nc.sync.dma_start(wt, w_d[128 * c:128 * (c + 1), :].bitcast(FPR))
                nc.tensor.matmul(kv_ps[0:1, :],
                                 ctxT_sb[:, c:c + 1], wt,
                                 start=(c == 0), stop=(c == NCC - 1))
            if wi == 0:
                nc.vector.tensor_copy(kctx_sb, kv_ps[0:1, :])
            else:
                nc.vector.memset(vctx_row[0:1, :], 0.0)
                vr3 = vctx_row.rearrange("p (h e) -> p h e", e=128)
                nc.vector.tensor_copy(
                    vr3[0:1, :, 0:D],
                    kv_ps[0:1, 0:NHEAD * D].rearrange("p (h e) -> p h e", e=D))
                nc.vector.tensor_copy(vr3[0:1, :, D:D + 1], ones16[0:1, 0:NHEAD])
                nc.sync.dma_start(vctx_row[64:65, :], vctx_row[0:1, :])

        # k_ctx^T per head; odd heads shifted to the 64-partition band
        kct_ps = pj_pool.tile([128, 512], FP, tag="pj", name="kct_ps")
        for h in range(NHEAD):
            nc.tensor.transpose(kct_ps[0:64, h:h + 1],
                                kctx_sb[0:1, 64 * h:64 * h + 64], ones11)
        nc.vector.tensor_copy(kct_sb, kct_ps[0:64, 0:NHEAD])
        nc.vector.tensor_copy(kct2, zeros8)
        for h in range(NHEAD):
            if h % 2 == 0:
                nc.vector.tensor_copy(kct2[0:64, h:h + 1], kct_sb[:, h:h + 1])
            else:
                nc.sync.dma_start(kct2[64:128, h:h + 1], kct_sb[:, h:h + 1])

        # ---- main per-chunk pipeline ----
        for c in range(NQC):
            lo, hi = QC * c, QC * (c + 1)
            kmax = (hi // 128)

            # projection: x^T then q^T for this chunk's 1024 queries
            for s in range(QC // 512):
                blk = lo + 512 * s
                xts = []
                for r in range(4):
                    xt = xn_pool.tile([128, CD], FPR, tag="xn", name="xn")
                    nc.sync.dma_start(xt, x_d[blk + 128 * r: blk + 128 * (r + 1), :].bitcast(FPR))
                    xts.append(xt)
                for c8 in range(NCC):
                    tps = pj_pool.tile([128, 512], FPR, tag="pj", name="tps")
                    for r in range(4):
                        nc.tensor.transpose(
                            tps[:, 128 * r:128 * (r + 1)],
                            xts[r][:, 128 * c8:128 * (c8 + 1)],
                            identR)
                    nc.vector.tensor_copy(xT_sb[c8], tps)
                for m in range(NMT):
                    qps = pj_pool.tile([128, 512], FP, tag="pj", name="qps")
                    for c8 in range(NCC):
                        nc.tensor.matmul(qps,
                                         wq_sb[c8][:, 128 * m:128 * (m + 1)],
                                         xT_sb[c8],
                                         start=(c8 == 0), stop=(c8 == NCC - 1))
                    nc.vector.tensor_copy(qkT[c][m][:, 512 * s:512 * (s + 1)], qps)

            # ctx score rows for this chunk: one matmul + one exp covers both
            # heads of the pair (kct2 is zero-banded per head, so the full-K
            # contraction against the pair's stacked q^T is exact); the odd
            # head's row is then DMA-shifted to partition 64 for the seeds
            for m in range(NMT):
                pps = sp_pool.tile([128, QC], FP, tag="sp", name="pps")
                for s2 in range(2):
                    nc.tensor.matmul(pps[0:2, 512 * s2:512 * (s2 + 1)],
                                     kct2[:, 2 * m:2 * m + 2],
                                     qkT[c][m][:, 512 * s2:512 * (s2 + 1)],
                                     start=True, stop=True)
                nc.scalar.activation(pcx[m][0:2, lo:hi], pps[0:2, :],
                                     mybir.ActivationFunctionType.Exp, scale=SCALE)
                nc.sync.dma_start(pcx[m][64:65, lo:hi], pcx[m][1:2, lo:hi])

            # v-blocks for this chunk's keys (all heads)
            for h in range(NHEAD):
                m, band = h // 2, 64 * (h % 2)
                vs3 = vsb[h][:, 0:NKB * (D + 1)].rearrange("p (a b) -> p a b", b=D + 1)
                vt = pj_pool.tile([128, 512], FPR, tag="pj", name="vt")
                for j in range(8):
                    nc.tensor.transpose(
                        vt[:, 64 * j:64 * (j + 1)],
                        qkT[c][m][band:band + 64, 128 * j:128 * j + 128],
                        id2R[band:band + 64, 0:64])
                nc.vector.tensor_copy(
                    vs3[:, 8 * c:8 * c + 8, 0:D],
                    vt.rearrange("p (j e) -> p j e", e=D))
                nc.vector.tensor_copy(vs3[:, 8 * c:8 * c + 8, D:D + 1], ones16[:, 0:8])

            # attention for this chunk, heads pipelined
            for h in range(NHEAD):
                m, band = h // 2, 64 * (h % 2)
                pu = pu_pool.tile([128, QC], FP, tag="pu", name="pu")
                # ctx contribution seeds the accumulator
                for s2 in range(2):
                    nc.tensor.matmul(pu[:, 512 * s2:512 * (s2 + 1)],
                                     vctx_row[band:band + 1, 128 * h:128 * h + 128],
                                     pcx[m][band:band + 1, lo + 512 * s2:lo + 512 * (s2 + 1)],
                                     start=True, stop=False)

                sp_tiles = {}

                def emit_S(kb, m=m, band=band, c=c, lo=lo, hi=hi, sp_tiles=sp_tiles):
                    i0 = 128 * (kb - 1)
                    clo = max(i0, lo)
                    spt = sp_pool.tile([128, QC], FP, tag="sp", name="spt")
                    kc, ko = i0 // QC, i0 % QC
                    keys = qkT[kc][m][band:band + 64, ko:ko + 128]
                    for (q0, q1) in _spans(clo, hi):
                        nc.tensor.matmul(spt[:, q0 - lo:q1 - lo],
                                         keys,
                                         qkT[c][m][band:band + 64, q0 - lo:q1 - lo],
                                         start=True, stop=True)
                    sp_tiles[kb] = (spt, clo)

                def emit_EU(kb, h=h, m=m, band=band, c=c, lo=lo, hi=hi,
                            kmax=kmax, pu=pu, sp_tiles=sp_tiles):
                    spt, clo = sp_tiles.pop(kb)
                    off = clo - lo
                    ptt = pt_pool.tile([128, QC], BF, tag="pt", name="ptt")
                    nc.scalar.activation(ptt[:, off:QC], spt[:, off:QC],
                                         mybir.ActivationFunctionType.Exp,
                                         scale=SCALE)
                    i0 = 128 * (kb - 1)
                    if i0 >= lo:
                        nc.vector.tensor_mul(ptt[:, off:off + 128],
                                             ptt[:, off:off + 128], tri)
                    for (q0, q1) in _spans(clo, hi):
                        # each 512-col PSUM bank closes when its last-touching
                        # key block writes it (later blocks only cover q >= i0)
                        nc.tensor.matmul(pu[:, q0 - lo:q1 - lo],
                                         vsb[h][:, 65 * (kb - 1):65 * (kb - 1) + 128],
                                         ptt[:, q0 - lo:q1 - lo],
                                         start=False,
                                         stop=(kb == min(kmax, q1 // 128)))

                emit_S(1)
                for kb in range(1, kmax + 1):
                    if kb + 1 <= kmax:
                        emit_S(kb + 1)
                    emit_EU(kb)

                # normalize: attnT = U[0:64] / U[64]; copy U out of PSUM first
                # so the pu slot frees fast
                u_sb = us_pool.tile([65, QC], FP, tag="us", name="u_sb")
                nc.vector.tensor_copy(u_sb, pu[0:65, :])
                r1 = rc_pool.tile([1, QC], FP, tag="rc1", name="r1")
                nc.vector.reciprocal_approx_fast(r1, u_sb[64:65, :])
                rb = rc_pool.tile([64, QC], FP, tag="rcb", name="rb")
                nc.gpsimd.partition_broadcast(rb, r1)
                nc.vector.tensor_mul(attnT[c][m][band:band + 64, :],
                                     u_sb[0:64, :], rb)

        # ---- output projection (chunk 0 then chunk 1; fills PE gaps during
        # attention of chunk 1 via the list scheduler) ----
        for c in range(NQC):
            for nb in range(QC // 128):
                for co in range(2):
                    py = pj_pool.tile([128, 512], FP, tag="pj", name="py")
                    for m in range(NMT):
                        nc.tensor.matmul(py,
                                         attnT[c][m][:, 128 * nb:128 * (nb + 1)],
                                         wo_sb[m][:, 512 * co:512 * (co + 1)],
                                         start=(m == 0), stop=(m == NMT - 1))
                    ysb = ys_pool.tile([128, 512], FP, tag="ysb", name="ysb")
                    nc.vector.tensor_copy(ysb, py)
                    nc.sync.dma_start(
                        y_d[QC * c + 128 * nb:QC * c + 128 * (nb + 1),
                            512 * co:512 * (co + 1)],
                        ysb)

    nc.compile()
    return nc


_NC = None


def _get_nc():
    global _NC
    if _NC is None:
        _NC = _build_nc()
    return _NC


def _shard(inputs):
    x = np.ascontiguousarray(np.asarray(inputs["x"], dtype=np.float32))
    context = np.ascontiguousarray(np.asarray(inputs["context"], dtype=np.float32))
    Wq = np.asarray(inputs["Wq"], dtype=np.float32)
    Wk = np.asarray(inputs["Wk"], dtype=np.float32)
    Wv = np.asarray(inputs["Wv"], dtype=np.float32)
    Wo = np.asarray(inputs["Wo"], dtype=np.float32)
    in_maps = []
    for c in range(8):
        b, g = c // 2, c % 2
        sl = slice(HD * g, HD * (g + 1))
        in_maps.append({
            "x": np.ascontiguousarray(x[b]),
            "wq": np.ascontiguousarray(Wq[:, sl]),
            "wk": np.ascontiguousarray(Wk[:, sl]),
            "wv": np.ascontiguousarray(Wv[:, sl]),
            "wo": np.ascontiguousarray(Wo[sl, :]),
            "ctx": np.ascontiguousarray(context[b:b + 1]),
        })
    return in_maps


def _run(inputs, trace=False, **kw):
    nc = _get_nc()
    in_maps = _shard(inputs)
    res = run_bass_kernel_spmd(nc, in_maps, list(range(8)), trace=trace, **kw)
    bo = np.asarray(inputs["bo"], dtype=np.float32)
    B = np.asarray(inputs["x"]).shape[0]
    y = np.empty((B, N, CD), dtype=np.float32)
    for b in range(B):
        y[b] = res.results[2 * b]["y"] + res.results[2 * b + 1]["y"] + bo
    return y, res


def kernel(**inputs):
    y, _ = _run(inputs)
    return y
